# revision 32
# baseline (speedup 1.0000x reference)
"""Trainium2 Bass kernel v3 for intra-segment KNN (K=64 neighbours + self).

Problem: coordinates [32768, 4] f32 split into 8 equal segments (events) of
4096 points; per point, find the 65 nearest points (incl. self) within its
segment, returning (idx int32 [32768,65], dist f32 [32768,65]) sorted by
ascending squared distance.

Sharding: one event per NeuronCore (8 cores), pure data parallel.

v3 design (597us -> 524us); DVE (the bottleneck engine, ~96% busy) runs
only the irreducible scan/extract work, everything else is offloaded:

  * Self-exclusion: the self point (d2=0) is always rank 1, so a Pool
    affine_select fills NEG_BIG on the diagonal block (j==p) before the
    group phase, and column 0 of the output is emitted directly (idx via
    a static iota, dist=0, both DMA'd straight from persistent tiles).
    The candidate phase then extracts ranks 2..65 = 64 values in 8
    rounds instead of 9 (-2 DVE passes over the candidate array).
  * M_PER_G=7: keep 7 (not 8) survivors per 128-wide group, shrinking
    the candidate array to 224.  Max8 still writes 8 values per group;
    the 8th slot of each group is excluded from the candidate phase via
    strided [P,32,7-of-8] views.  Measured accuracy cost: dist rel err
    2.1e-4 -> 1.85e-3 on hardware, far below the 2e-2 gate.
  * Csave stores the FULL local column per slot ((Cv & 0x7F) | g*128 --
    offset and group-base bits are disjoint so one STT assembles it),
    letting the Pool rank->slot scatter write neighbour columns straight
    into the output tile with no index arithmetic afterwards.
  * Embed is one STT over [P,32,128]; tile 0 embeds per psum-half so the
    first group scans start ~5us earlier.
  * Int format conversions (i32->i16) and the W2-1 bias ride the Scalar
    engine (fp32-internal, exact for values <= 4095); Pool does the two
    local_scatters + the diagonal affine_select.
  * out_idx is int16 on device (segment-local columns < 4096); the host
    widens to int32 while adding the segment base.

Per-core DVE per 128-row tile (cost model): embed 4.33us + 32x Max8
6.21us + candidate phase (8 Max + 7 MatchReplace over 224) 4.41us +
Csave/re-embed STTs 0.59us + V&0xFF 0.09us = 15.6us; 32 tiles + ~21us
of startup/tail latency = 524us.
"""

import numpy as np

S = 4096          # points per segment
D = 4             # coordinate dims
B = 8             # segments / cores
K1 = 65           # neighbours incl. self
P = 128           # partitions
NT = S // P       # 32 row tiles
GW = 128          # group width (columns per group)
NG = S // GW      # 32 groups
M_PER_G = 7       # survivors kept per group (of the 8 Max8 emits)
CW = NG * M_PER_G # active candidate count (224)
NR = 8            # extraction rounds (8*8 = 64 = ranks 2..65)
RW = NR * 8       # 64
NEG_BIG = -3.0e38 # "minus infinity" replacement value

_NC_CACHE = {}


def _build_nc(nt=NT):
    import concourse.bacc as bacc
    import concourse.mybir as mybir
    from concourse import bass
    from concourse.tile import TileContext

    fp32 = mybir.dt.float32
    i16 = mybir.dt.int16
    i32 = mybir.dt.int32
    Alu = mybir.AluOpType
    Act = mybir.ActivationFunctionType

    nc = bacc.Bacc(None, target_bir_lowering=False, debug=False)

    coords = nc.dram_tensor("coords", [S, D], fp32, kind="ExternalInput")
    out_dist = nc.dram_tensor("out_dist", [nt * P, K1], fp32,
                              kind="ExternalOutput")
    out_idx = nc.dram_tensor("out_idx", [nt * P, K1], i16,
                             kind="ExternalOutput")

    def v7(ap):
        """[P, NG*8] (8-slot groups) -> strided [P, NG, M_PER_G] view."""
        return ap.rearrange("p (g s) -> p g s", s=8)[:, :, :M_PER_G]

    with TileContext(nc) as tc:
        with (
            tc.tile_pool(name="const", bufs=1) as cpool,
            tc.tile_pool(name="nk", bufs=2) as nkpool,
            tc.tile_pool(name="cand", bufs=2) as candpool,
            tc.tile_pool(name="small", bufs=3) as spool,
            tc.tile_pool(name="outs", bufs=3) as opool,
            tc.tile_pool(name="psum", bufs=2, space="PSUM") as ppool,
        ):
            # ---------------- persistent tensors ----------------
            rhs8 = cpool.tile([8, S], fp32)     # rows 0-3: c^T, rows 4-7: (c^T)^2
            lhsT8 = cpool.tile([8, S], fp32)    # rows 0-3: 2*c^T, rows 4-7: -1
            offpat = cpool.tile([P, GW], i32)   # 0..GW-1 (broadcast over groups)
            ct_all = cpool.tile([P, NT * D], fp32)  # coords tile-major
            sq_all = cpool.tile([P, NT * D], fp32)  # squares tile-major
            sqr_neg = cpool.tile([P, NT], fp32) # -|c_r|^2 per row, per tile col

            nc.gpsimd.memset(lhsT8, -1.0)   # Pool is free at t=0
            nc.gpsimd.iota(offpat, [[1, GW]], base=0, channel_multiplier=0)
            # int32 scalar constants (bitvec ops need int-typed operands)
            cm128 = cpool.tile([P, 1], i32)   # 0xFFFFFF80
            cm256 = cpool.tile([P, 1], i32)   # 0xFFFFFF00
            nc.gpsimd.memset(cm128, -128)
            nc.gpsimd.memset(cm256, -256)
            # compact slot ids 0..CW-1 laid out at the 8-slot-group positions
            # (value at physical slot g*8+k is g*M_PER_G+k; the k=7 slots are
            # never read)
            slotpat = cpool.tile([P, NG * 8], i32)
            nc.gpsimd.iota(slotpat, [[M_PER_G, NG], [1, 8]], base=0,
                           channel_multiplier=0)
            kio1 = cpool.tile([P, RW], i16)     # 1..RW
            nc.gpsimd.iota(kio1, [[1, RW]], base=1, channel_multiplier=0)
            # group column base per physical slot: tg32[g*8+k] = g*GW
            tg32 = cpool.tile([P, NG * 8], i32)
            nc.gpsimd.iota(tg32, [[GW, NG], [0, 8]], base=0,
                           channel_multiplier=0)
            # per-tile column of self: col0_all[p, t] = t*P + p
            col0_all = cpool.tile([P, NT], i16)
            nc.gpsimd.iota(col0_all, [[P, NT]], base=0, channel_multiplier=1)
            zero1f = cpool.tile([P, 1], fp32)   # dist column 0 (self)
            nc.gpsimd.memset(zero1f, 0.0)
            c255 = cpool.tile([P, 1], i32)
            nc.gpsimd.memset(c255, 255)
            c127 = cpool.tile([P, 1], i32)
            nc.gpsimd.memset(c127, 127)
            negone = cpool.tile([P, 1], fp32)   # ScalarE bias for W2m = W2-1
            nc.gpsimd.memset(negone, -1.0)

            from concourse import library_config

            # ---------------- prologue (bulk, no PE transposes) ----------
            # PE p-state warmup: dummy matmuls keep the tensor engine busy
            # through its 3us clock ramp so tile 0's real matmuls run at
            # full speed
            dum = cpool.tile([8, 512], fp32)
            nc.gpsimd.memset(dum, 1.0)
            for w in range(3):
                pdum = ppool.tile([P, 512], fp32, tag="pdum")
                nc.tensor.matmul(pdum, dum[:, 0:P], dum,
                                 start=True, stop=True)

            # rhs8 rows 0-3 <- coords^T via transpose DMA (AP swap), chunked
            # so the first matmuls can start early
            # per-row |c_r|^2 first: it feeds every tile's ScalarE bias
            # (high priority: tile 0's first bias-act sits on this chain)
            with tc.high_priority():
                nc.scalar.dma_start(
                    ct_all.rearrange("p (t c) -> p t c", c=D),
                    coords.rearrange("(t p) c -> p t c", p=P))
                nc.scalar.activation(sq_all, ct_all, Act.Square)
                nc.vector.tensor_reduce(
                    sqr_neg.unsqueeze(-1),
                    sq_all.rearrange("p (t c) -> p t c", c=D),
                    axis=mybir.AxisListType.X, op=Alu.add, negate=True,
                )

            sq4 = cpool.tile([D, S], fp32)
            PCH = 1024
            dma_queues = [nc.sync, nc.scalar]
            for ch in range(S // PCH):
                cc = slice(ch * PCH, (ch + 1) * PCH)
                q = dma_queues[ch % len(dma_queues)]
                q.dma_start(rhs8[0:D, cc],
                            coords[cc, :].rearrange("a b -> b a"))
                nc.scalar.activation(sq4[:, cc], rhs8[0:D, cc], Act.Square)
                q.dma_start(rhs8[D:2 * D, cc], sq4[:, cc])
                # 2*c^T on DVE (idle during prologue; keeps ScalarE short)
                nc.vector.tensor_scalar_mul(lhsT8[0:D, cc], rhs8[0:D, cc], 2.0)

            # local_scatter lives in gpsimd ucode library 7; load it once
            nc.gpsimd.load_library(library_config.local_scatter)

            # ---------------- main loop over row tiles ----------------
            def _stage_b1(pn):
                W2m = spool.tile([P, CW], i16, tag="W2m")
                nc.scalar.activation(W2m, pn["W2"], Act.Identity, bias=negone)
                # scatter each winning slot's local column straight into the
                # output tile's columns 1..64 (rank order)
                idxb = opool.tile([P, RW], i16, tag="idxb")
                nc.gpsimd.local_scatter(
                    idxb, pn["Csave"], W2m, channels=P, num_elems=RW,
                    num_idxs=CW)
                pn["idxb"] = idxb

            def _stage_b2(pn):
                dist64 = opool.tile([P, RW], fp32, tag="dist64")
                nc.scalar.activation(dist64, pn["V"], Act.Relu, scale=-1.0)
                t, cs = pn["t"], pn["cs"]
                nc.sync.dma_start(out_dist[cs, 0:1], zero1f)
                nc.sync.dma_start(out_dist[cs, 1:K1], dist64)
                nc.sync.dma_start(out_idx[cs, 0:1], col0_all[:, t:t + 1])
                nc.sync.dma_start(out_idx[cs, 1:K1], pn["idxb"])

            pending = None
            HB = 1024               # psum half-block columns
            for t in range(nt):
                cs = slice(t * P, (t + 1) * P)
                nk = nkpool.tile([P, S], fp32, tag="nk")
                for h in range(S // HB):
                    pshalf = ppool.tile([P, HB], fp32, tag="pshalf")
                    for m in range(HB // 512):
                        col0 = h * HB + m * 512
                        nc.tensor.matmul(
                            pshalf[:, m * 512:(m + 1) * 512],
                            lhsT8[:, cs],
                            rhs8[:, col0:col0 + 512],
                            start=True, stop=True,
                        )
                    # nk = psum - |c_r|^2 = -d2 (key quantum tracks d2)
                    nc.scalar.activation(
                        nk[:, h * HB:(h + 1) * HB], pshalf,
                        Act.Identity, bias=sqr_neg[:, t:t + 1],
                    )

                # ---- embed 7-bit column offset into low mantissa bits ----
                # ekey = (nk & 0xFFFFFF80) | (j % GW)   (in-place, int32 view)
                nki = nk.bitcast(i32)

                def emit_embed(ha, hb):
                    sl = slice(ha * HB, hb * HB)
                    ngs = (hb - ha) * (HB // GW)
                    nc.vector.scalar_tensor_tensor(
                        nki[:, sl].rearrange("p (g w) -> p g w", w=GW),
                        nki[:, sl].rearrange("p (g w) -> p g w", w=GW),
                        cm128,
                        offpat.unsqueeze(1).broadcast_to((P, ngs, GW)),
                        op0=Alu.bitwise_and, op1=Alu.bitwise_or,
                    )

                def emit_dk():
                    # self-exclusion: fill NEG_BIG on the diagonal of the
                    # tile's own column block (j - p == 0), on Pool
                    dk = spool.tile([P, GW], fp32, tag="dk")
                    nc.gpsimd.affine_select(
                        dk, nk[:, t * GW:(t + 1) * GW], [[1, GW]],
                        compare_op=Alu.not_equal, fill=NEG_BIG,
                        base=0, channel_multiplier=-1,
                    )
                    return dk

                # ---- group phase: top-8 of each 128-wide group ----
                # (group t -- the diag-killed copy -- is done last so the
                # Pool affine_select never stalls DVE; the previous tile's
                # scatter-dependent decode tail is emitted mid-phase)
                Cv = candpool.tile([P, NG * 8], fp32, tag="Cv")
                GPH = HB // GW          # groups per psum half
                if t == 0:
                    # tile 0 has nothing to hide its latency behind: embed
                    # and group-scan each psum half as soon as it lands
                    emit_embed(0, 1)
                    dk = emit_dk()
                    for h in range(1, S // HB):
                        for g in range(h * GPH - GPH, h * GPH):
                            if g != t:
                                nc.vector.max(Cv[:, g * 8:g * 8 + 8],
                                              nk[:, g * GW:(g + 1) * GW])
                        emit_embed(h, h + 1)
                    for g in range((S // HB - 1) * GPH, NG):
                        if g != t:
                            nc.vector.max(Cv[:, g * 8:g * 8 + 8],
                                          nk[:, g * GW:(g + 1) * GW])
                    nc.vector.max(Cv[:, t * 8:t * 8 + 8], dk)
                else:
                    emit_embed(0, S // HB)
                    dk = emit_dk()
                    order = [g for g in range(NG) if g != t] + [t]
                    for i, g in enumerate(order):
                        src = dk if g == t else nk[:, g * GW:(g + 1) * GW]
                        nc.vector.max(Cv[:, g * 8:g * 8 + 8], src)
                        if i == 10 and pending is not None:
                            _stage_b1(pending)
                        if i == 24 and pending is not None:
                            _stage_b2(pending)
                            pending = None

                Cv7f = v7(Cv)
                Cv7i = v7(Cv.bitcast(i32))
                # ---- save per-slot LOCAL COLUMNS: (Cv & 0x7F) | g*GW ----
                # (offset bits 0..6 and group-base bits 7..11 are disjoint,
                # so the OR assembles the full column id in one STT)
                Csave32 = spool.tile([P, CW], i32, tag="Csave32")
                nc.vector.scalar_tensor_tensor(
                    Csave32.rearrange("p (g s) -> p g s", s=M_PER_G),
                    Cv7i, c127, v7(tg32),
                    op0=Alu.bitwise_and, op1=Alu.bitwise_or,
                )
                Csave = spool.tile([P, CW], i16, tag="Csave")
                nc.scalar.activation(Csave, Csave32, Act.Identity)
                # Cv = (Cv & ~0xFF) | slot -- low 8 bits now hold the compact
                # slot id, so the extraction below needs no MaxIndex at all
                nc.vector.scalar_tensor_tensor(
                    Cv7i, Cv7i, cm256, v7(slotpat),
                    op0=Alu.bitwise_and, op1=Alu.bitwise_or,
                )

                # ---- C phase: ranks 2..65 (slots ride in the low bits) ----
                V = spool.tile([P, RW], fp32, tag="V")
                for r in range(NR):
                    v8 = V[:, r * 8:(r + 1) * 8]
                    nc.vector.max(v8, Cv7f)
                    if r + 1 < NR:
                        nc.vector.match_replace(Cv7f, v8, Cv7f, NEG_BIG)

                # ---- decode: c = V & 0xFF; offsets and group bases are
                # recovered on Pool via rank->slot scatters ----
                qwin32 = spool.tile([P, RW], i32, tag="qwin32")
                nc.vector.tensor_scalar(
                    qwin32, V.bitcast(i32), c255, None,
                    op0=Alu.bitwise_and,
                )
                qwin = spool.tile([P, RW], i16, tag="qwin")
                nc.scalar.activation(qwin, qwin32, Act.Identity)
                W2 = spool.tile([P, CW], i16, tag="W2")
                nc.gpsimd.local_scatter(
                    W2, kio1, qwin, channels=P, num_elems=CW, num_idxs=RW)
                pending = dict(W2=W2, Csave=Csave, V=V, cs=cs, t=t)

            _stage_b1(pending)
            _stage_b2(pending)
            pending = None

    nc.finalize()
    return nc


def _get_nc():
    if "nc" not in _NC_CACHE:
        _NC_CACHE["nc"] = _build_nc()
    return _NC_CACHE["nc"]


def _numpy_fallback(coordinates, row_splits):
    """Pure-numpy replica of the reference (used only on unexpected shapes)."""
    nB = int(row_splits.shape[0] - 1)
    N, nD = coordinates.shape
    nS = N // nB
    c = coordinates.reshape(nB, nS, nD).astype(np.float32)
    sq = np.sum(c * c, axis=-1)
    d2 = sq[:, :, None] + sq[:, None, :] - 2.0 * np.einsum(
        "bsd,btd->bst", c, c)
    d2 = np.maximum(d2, 0.0).astype(np.float32)
    k1 = min(K1, nS)
    idx = np.argsort(d2, axis=-1, kind="stable")[:, :, :k1]
    dist = np.take_along_axis(d2, idx, axis=-1)
    idx = idx + (np.arange(nB, dtype=np.int32) * nS)[:, None, None]
    return (idx.reshape(N, k1).astype(np.int32),
            dist.reshape(N, k1).astype(np.float32))


def kernel(coordinates, row_splits):
    coordinates = np.ascontiguousarray(coordinates, dtype=np.float32)
    rs = np.asarray(row_splits)
    expected_rs = np.arange(B + 1, dtype=np.int64) * S
    if coordinates.shape != (B * S, D) or rs.shape != (B + 1,) or \
            not np.array_equal(rs.astype(np.int64), expected_rs):
        return _numpy_fallback(coordinates, rs)

    from concourse import bass_utils

    nc = _get_nc()
    in_maps = [
        {"coords": coordinates[b * S:(b + 1) * S]} for b in range(B)
    ]
    res = bass_utils.run_bass_kernel_spmd(nc, in_maps, core_ids=list(range(B)))
    idx = np.concatenate(
        [res.results[b]["out_idx"].astype(np.int32) + np.int32(b * S)
         for b in range(B)], axis=0
    ).astype(np.int32)
    dist = np.concatenate(
        [res.results[b]["out_dist"] for b in range(B)], axis=0
    ).astype(np.float32)
    return idx, dist
